# revision 6
# baseline (speedup 1.0000x reference)
"""Multi-head attention (b=4, n=2048, dim=768, 12 heads) on 8 TRN2 NeuronCores.

Sharding: core c handles batch c//2 and head-group c%2 (6 of 12 heads).  Each
core computes its heads' contribution projected through its slice of Wo and
returns a partial [2048, 768] f32 output; the host sums core pairs and adds
the bias.  No on-device collectives needed.

Per-core kernel (all TensorE data bf16, accumulation f32):
  P1: KT/QT = W^T x^T feature-major per head-pair f (rows 0:64 = head 2f,
      64:128 = head 2f+1), V token-major with a ones column at 64 per head
      block (OP row 64 = softmax denominator for free).
  P2: per (head-pair f, i-block, j-chunk c): scores TRANSPOSED ST[j,i] = K Q^T
      (K=64) for BOTH heads into one [128, 1024] PSUM tile; the two score
      matmuls use PE row-halves 0:64 / 64:128 (tile_position), which the PE
      executes concurrently -> 2x throughput at K=64; score emission is
      batched in j-chunk pairs to reduce PE mode-turnaround.  Each ST tile's
      exp is split ACROSS BOTH engines concurrently: ACT runs Exp on columns
      [0:SPL] (bf16 out), DVE covers [SPL:1024] with a Schraudolph bit trick
      (one tensor_scalar: int16(round(s*log2(e)*128 + 16248.6)) whose bits
      ARE bf16(exp(s)); ~1.8% rms per weight, washes out in softmax).
      attnV lags 4 j-chunks behind the scores (hiding exp latency),
      accumulates OP[f,i] per head in PSUM over the 16 j-chunks, and drains
      2 per step once scores finish (taper).  PSUM: 3x2-bank score tiles
      + 2 accumulator banks = 8.
      Epilogue per (f, i-block): reciprocal of OP row 64 -> linv [1,512],
      DMA stride-0 partition-broadcast to [64,512], one DVE multiply
      normalizes OP features (PSUM f32 -> SBUF bf16) into OPn[f][ib]
      [128,512] feature-major (rows 0:64 head 2f, 64:128 head 2f+1).
  P3: transpose-free output projection: OPn tiles ARE the K=128 lhsT slices
      (features on partitions), so out[i,:] accumulates 3 matmuls per
      128-token block directly; PSUM -> SBUF copy on ACT; DMA out.
"""
import os
import sys
import types
import numpy as np
import ml_dtypes

B, N, DIM = 4, 2048, 768
HEADS, DH = 12, 64
HPC = 6                # heads per core
FPC = HPC * DH         # 384 features per core
NCORES = 8
KC = DIM // 128        # 6 contraction chunks
FT = 3                 # head-pairs per core
NT = N // 128          # 16 j-chunks of 128
IBS = 512              # i-block size
IB = N // IBS          # 4 i-blocks
BF16 = ml_dtypes.bfloat16

SC = 0.125
C1B = SC * 184.66496   # DVE bits: log2(e)*128 * logit scale
C2B = 16256.0 - 7.4    # bf16 exponent bias - Schraudolph sigma
SPL = 512              # exp split point: ACT does [0:SPL], DVE [SPL:1024]

_cache = {}
last_exec_time_ns = None


def _install_ntff_hook():
    try:
        import antenv.axon_hooks  # noqa: F401
        return
    except ImportError:
        pass
    from trn_agent_boot.trn_boot import _ntff_profile_via_ctypes
    hook = _ntff_profile_via_ctypes('/opt/axon/libaxon_pjrt.so')
    mod = types.ModuleType('antenv.axon_hooks')
    mod.get_axon_ntff_profile_hook = lambda: hook
    import antenv
    sys.modules['antenv.axon_hooks'] = mod
    antenv.axon_hooks = mod


def _build_nc():
    from contextlib import ExitStack
    from concourse import bacc
    import concourse.mybir as mybir
    from concourse.tile import TileContext
    from concourse import bass
    from concourse.bass import broadcast_tensor_aps  # noqa: F401

    dt = mybir.dt
    EXP = mybir.ActivationFunctionType.Exp
    LN = mybir.ActivationFunctionType.Ln
    MUL, ADD = mybir.AluOpType.mult, mybir.AluOpType.add

    nc = bacc.Bacc("TRN2", target_bir_lowering=False, debug=False,
                   num_devices=NCORES)
    xT = nc.dram_tensor("xT", [DIM, N], dt.bfloat16, kind="ExternalInput").ap()
    wq = nc.dram_tensor("wq", [DIM, FPC], dt.bfloat16, kind="ExternalInput").ap()
    wk = nc.dram_tensor("wk", [DIM, FPC], dt.bfloat16, kind="ExternalInput").ap()
    wv = nc.dram_tensor("wv", [DIM, FPC], dt.bfloat16, kind="ExternalInput").ap()
    wo = nc.dram_tensor("wo", [FPC, DIM], dt.bfloat16, kind="ExternalInput").ap()
    out = nc.dram_tensor("out", [N, DIM], dt.float32, kind="ExternalOutput").ap()

    with TileContext(nc) as tc, ExitStack() as ctx:
        inp = ctx.enter_context(tc.tile_pool(name="inp", bufs=1))
        xts2 = [[inp.tile([128, N // 2], dt.bfloat16, tag=f"xt{k}_{hf}",
                          name=f"xt{k}_{hf}") for hf in range(2)]
                for k in range(KC)]
        wqs = [inp.tile([128, FPC], dt.bfloat16, tag=f"wq{k}", name=f"wq{k}")
               for k in range(KC)]
        wks = [inp.tile([128, FPC], dt.bfloat16, tag=f"wk{k}", name=f"wk{k}")
               for k in range(KC)]
        wvs = [inp.tile([128, FPC], dt.bfloat16, tag=f"wv{k}", name=f"wv{k}")
               for k in range(KC)]
        wos = [inp.tile([128, DIM], dt.bfloat16, tag=f"wo{f}", name=f"wo{f}")
               for f in range(FT)]
        for k in range(KC):
            nc.sync.dma_start(out=xts2[k][0][:],
                              in_=xT[k * 128:(k + 1) * 128, 0:N // 2])
            nc.scalar.dma_start(out=wvs[k][:], in_=wv[k * 128:(k + 1) * 128, :])
        for k in range(KC):
            nc.sync.dma_start(out=xts2[k][1][:],
                              in_=xT[k * 128:(k + 1) * 128, N // 2:N])
        for k in range(KC):
            nc.sync.dma_start(out=wks[k][:], in_=wk[k * 128:(k + 1) * 128, :])
            nc.sync.dma_start(out=wqs[k][:], in_=wq[k * 128:(k + 1) * 128, :])
        for f in range(FT):
            nc.scalar.dma_start(out=wos[f][:], in_=wo[f * 128:(f + 1) * 128, :])

        kqv = ctx.enter_context(tc.tile_pool(name="kqv", bufs=1))
        KT = [kqv.tile([128, N], dt.bfloat16, tag=f"kt{f}", name=f"kt{f}")
              for f in range(FT)]
        QT = [kqv.tile([128, N], dt.bfloat16, tag=f"qt{f}", name=f"qt{f}")
              for f in range(FT)]
        VP = [kqv.tile([128, HPC * 128], dt.bfloat16, tag=f"vp{t}", name=f"vp{t}")
              for t in range(NT)]
        opnb = ctx.enter_context(tc.tile_pool(name="opnb", bufs=1))
        OPn = [[opnb.tile([128, IBS], dt.bfloat16, tag=f"opn{f}_{ib}",
                          name=f"opn{f}_{ib}") for ib in range(IB)]
               for f in range(FT)]

        # ---- P1: projections ----
        for t in range(NT):
            nc.vector.memset(
                VP[t].rearrange("p (h c) -> p h c", c=128)[:, :, 64:65], 1.0)
        with tc.tile_pool(name="p1ps", bufs=6, space="PSUM") as p1:
            for t in range(NT):
                ps = p1.tile([128, FPC], dt.float32, tag="p1", name=f"vps{t}")
                for k in range(KC):
                    nc.tensor.matmul(
                        ps[:],
                        lhsT=xts2[k][t // 8][:, (t % 8) * 128:(t % 8 + 1) * 128],
                        rhs=wvs[k][:], start=(k == 0), stop=(k == KC - 1))
                nc.vector.tensor_copy(
                    VP[t].rearrange("p (h c) -> p h c", c=128)[:, :, 0:64],
                    ps.rearrange("p (h c) -> p h c", c=64))
            for W, DST in ((wks, KT), (wqs, QT)):
                for f in range(FT):
                    for q in range(N // 512):
                        ps = p1.tile([128, 512], dt.float32, tag="p1",
                                     name=f"kqps{f}_{q}")
                        for k in range(KC):
                            nc.tensor.matmul(
                                ps[:], lhsT=W[k][:, f * 128:(f + 1) * 128],
                                rhs=xts2[k][q // 2][:, (q % 2) * 512:
                                                    (q % 2 + 1) * 512],
                                start=(k == 0), stop=(k == KC - 1))
                        nc.scalar.copy(DST[f][:, q * 512:(q + 1) * 512], ps[:])

        # ---- P2: paired scores + alternating-engine exp + attnV ----
        LAG = 4
        with tc.tile_pool(name="p2st", bufs=2, space="PSUM") as p2st, \
                tc.tile_pool(name="p2op", bufs=4, space="PSUM") as p2op, \
                tc.tile_pool(name="exbp", bufs=LAG + 2) as exbp, \
                tc.tile_pool(name="linp", bufs=4) as linp, \
                tc.tile_pool(name="linbp", bufs=4) as linbp:
            for f in range(FT):
                for ib in range(IB):
                    opE = p2op.tile([128, IBS], dt.float32, tag="op",
                                    name=f"opE{f}_{ib}")
                    opO = p2op.tile([128, IBS], dt.float32, tag="op",
                                    name=f"opO{f}_{ib}")
                    pend = []
                    scq = []
                    for c in range(NT + 2):
                        if c < NT:
                            st = p2st.tile([128, 2 * IBS], dt.float32,
                                           tag="st", name=f"st{f}_{ib}_{c}")
                            scq.append((st, c))
                            if c % 2 == 1:
                                for stx, cc in scq:
                                    nc.tensor.matmul(
                                        stx[:, 0:IBS],
                                        lhsT=KT[f][0:64,
                                                   cc * 128:(cc + 1) * 128],
                                        rhs=QT[f][0:64,
                                                  ib * IBS:(ib + 1) * IBS],
                                        start=True, stop=True)
                                    nc.tensor.matmul(
                                        stx[:, IBS:2 * IBS],
                                        lhsT=KT[f][64:128,
                                                   cc * 128:(cc + 1) * 128],
                                        rhs=QT[f][64:128,
                                                  ib * IBS:(ib + 1) * IBS],
                                        start=True, stop=True)
                                for stx, cc in scq:
                                    ex = exbp.tile([128, 2 * IBS], dt.bfloat16,
                                                   tag="exb",
                                                   name=f"ex{f}_{ib}_{cc}")
                                    nc.scalar.activation(ex[:, 0:SPL],
                                                         stx[:, 0:SPL],
                                                         EXP, scale=SC)
                                    nc.vector.tensor_scalar(
                                        ex[:].bitcast(dt.int16)
                                        [:, SPL:2 * IBS],
                                        stx[:, SPL:2 * IBS], C1B, C2B,
                                        MUL, ADD)
                                    pend.append((ex, cc))
                                scq = []
                        ndr = (1 if c >= LAG else 0) + (1 if c >= NT else 0)
                        for _ in range(ndr):
                            ex, jc = pend.pop(0)
                            nc.tensor.matmul(
                                opE[:],
                                lhsT=VP[jc][:, (2 * f) * 128:
                                             (2 * f + 1) * 128],
                                rhs=ex[:, 0:IBS],
                                start=(jc == 0), stop=(jc == NT - 1))
                            nc.tensor.matmul(
                                opO[:],
                                lhsT=VP[jc][:, (2 * f + 1) * 128:
                                             (2 * f + 2) * 128],
                                rhs=ex[:, IBS:2 * IBS],
                                start=(jc == 0), stop=(jc == NT - 1))
                    # epilogue: normalize OP features into OPn (feature-major)
                    lnE = linp.tile([1, IBS], dt.float32, tag="ln",
                                    name=f"lnE{f}_{ib}")
                    lnO = linp.tile([1, IBS], dt.float32, tag="ln",
                                    name=f"lnO{f}_{ib}")
                    liE = linp.tile([1, IBS], dt.float32, tag="li",
                                    name=f"liE{f}_{ib}")
                    liO = linp.tile([1, IBS], dt.float32, tag="li",
                                    name=f"liO{f}_{ib}")
                    # 1/l = exp(-ln(l)) on ACT: ~1e-5 rel err, keeps DVE free
                    nc.scalar.activation(lnE[:], opE[64:65, :], LN)
                    nc.scalar.activation(lnO[:], opO[64:65, :], LN)
                    nc.scalar.activation(liE[:], lnE[:], EXP, scale=-1.0)
                    nc.scalar.activation(liO[:], lnO[:], EXP, scale=-1.0)
                    lbE = linbp.tile([64, IBS], dt.float32, tag="lb",
                                     name=f"lbE{f}_{ib}")
                    lbO = linbp.tile([64, IBS], dt.float32, tag="lb",
                                     name=f"lbO{f}_{ib}")
                    for li, lb in ((liE, lbE), (liO, lbO)):
                        nc.gpsimd.partition_broadcast(lb[:], li[:], channels=64)
                    nc.vector.tensor_mul(OPn[f][ib][0:64, :],
                                         opE[0:64, :], lbE[:])
                    nc.vector.tensor_mul(OPn[f][ib][64:128, :],
                                         opO[0:64, :], lbO[:])

        # ---- P3: output projection (transpose-free) ----
        with tc.tile_pool(name="p3pp", bufs=4, space="PSUM") as p3pp, \
                tc.tile_pool(name="outst", bufs=4) as outst:
            for isub in range(NT):
                ib, col = isub // 4, (isub % 4) * 128
                ob = outst.tile([128, DIM], dt.float32, tag="ob",
                                name=f"ob{isub}")
                for half in range(2):
                    pp = p3pp.tile([128, DIM // 2], dt.float32, tag="pp",
                                   name=f"pp{isub}_{half}")
                    for f in range(FT):
                        nc.tensor.matmul(
                            pp[:], lhsT=OPn[f][ib][:, col:col + 128],
                            rhs=wos[f][:, half * 384:(half + 1) * 384],
                            start=(f == 0), stop=(f == FT - 1))
                    nc.scalar.copy(ob[:, half * 384:(half + 1) * 384], pp[:])
                nc.sync.dma_start(out=out[isub * 128:(isub + 1) * 128, :],
                                  in_=ob[:])

    nc.finalize()
    return nc


def _get_nc():
    if "nc" not in _cache:
        _cache["nc"] = _build_nc()
    return _cache["nc"]


def kernel(x, Wq, Wk, Wv, Wo, bo):
    global last_exec_time_ns
    x = np.asarray(x, dtype=np.float32)
    Wq = np.asarray(Wq, dtype=np.float32)
    Wk = np.asarray(Wk, dtype=np.float32)
    Wv = np.asarray(Wv, dtype=np.float32)
    Wo = np.asarray(Wo, dtype=np.float32)
    bo = np.asarray(bo, dtype=np.float32)

    trace = bool(os.environ.get("BASS_KERNEL_TRACE"))
    if trace:
        _install_ntff_hook()
        import concourse.bass_utils as bass_utils
        bass_utils.upload_artifacts = lambda tmpdir: tmpdir

    nc = _get_nc()
    in_maps = []
    for c in range(NCORES):
        bi, hg = divmod(c, 2)
        s = slice(hg * FPC, (hg + 1) * FPC)
        in_maps.append({
            "xT": np.ascontiguousarray(x[bi].T).astype(BF16),
            "wq": np.ascontiguousarray(Wq[:, s]).astype(BF16),
            "wk": np.ascontiguousarray(Wk[:, s]).astype(BF16),
            "wv": np.ascontiguousarray(Wv[:, s]).astype(BF16),
            "wo": np.ascontiguousarray(Wo[s, :]).astype(BF16),
        })

    from concourse.bass_utils import run_bass_kernel_spmd
    res = run_bass_kernel_spmd(nc, in_maps, list(range(NCORES)), trace=trace)
    last_exec_time_ns = res.exec_time_ns

    parts = [res.results[c]["out"] for c in range(NCORES)]
    full = np.empty((B, N, DIM), np.float32)
    for bi in range(B):
        full[bi] = parts[2 * bi] + parts[2 * bi + 1] + bo[None, :]
    return full
